# revision 1
# baseline (speedup 1.0000x reference)
"""FWHT (normalized Walsh-Hadamard transform) over the last dim of x[4,4096,4096].

Strategy: rows are independent -> shard 16384 rows across 8 NeuronCores
(2048 rows each).  Per row, H_4096 = H_32 (x) H_128 (Sylvester Kronecker
factorization), so with the row viewed as X[c, kc] (c in [0,32), kc in [0,128)):

    y[c'*128+kc'] = (1/64) * sum_{c,kc} H32[c',c] H128[kc',kc] X[c,kc]

Per 256-row block on a core:
  T0:  TensorE transposes [r | k-chunk] -> Z[kc | r] per c-chunk
  A :  matmul, stationary H128/8, moving Z_c            -> B[kc' | r] per c
  T1:  TensorE transposes gather (rr,c) slices of B     -> Z2[(rr,c) | kc']
  B :  matmul, stationary kron(I4,H32)/8, moving Z2     -> C[(rr',c') | (u,kc')]
  DMA out with 512B-contiguous runs (kc' fastest).
"""

import numpy as np

try:
    import concourse.bass as bass  # noqa: F401
except ImportError:
    import sys

    sys.path.insert(0, "/opt/trn_rl_repo")

from concourse import bacc, bass, bass_utils, tile
from concourse import mybir

F32 = mybir.dt.float32
F32R = mybir.dt.float32r

N_CORES = 8
ROWS_PER_CORE = 2048
DIM = 4096
N_BLOCKS = 8       # blocks of 256 rows per core
BLOCK_ROWS = 256


def _hadamard(n: int) -> np.ndarray:
    h = np.array([[1.0]], dtype=np.float64)
    while h.shape[0] < n:
        h = np.block([[h, h], [h, -h]])
    return h


def _constants():
    h128 = (_hadamard(128) * 0.125).astype(np.float32)
    m32 = (np.kron(np.eye(4), _hadamard(32)) * 0.125).astype(np.float32)
    ident = np.eye(128, dtype=np.float32)
    return h128, m32, ident


def build_program(repeat: int = 1, hw_loop: bool = False,
                  dummy_out_dma: bool = False):
    nc = bacc.Bacc(
        "TRN2",
        target_bir_lowering=False,
        debug=False,
        enable_asserts=False,
    )

    x_d = nc.dram_tensor("x", [ROWS_PER_CORE, DIM], F32R, kind="ExternalInput").ap()
    h128_d = nc.dram_tensor("h128", [128, 128], F32R, kind="ExternalInput").ap()
    m32_d = nc.dram_tensor("m32", [128, 128], F32R, kind="ExternalInput").ap()
    id_d = nc.dram_tensor("ident", [128, 128], F32R, kind="ExternalInput").ap()
    y_d = nc.dram_tensor("y", [ROWS_PER_CORE, DIM], F32, kind="ExternalOutput").ap()

    with tile.TileContext(nc) as tc:
        with (
            tc.tile_pool(name="const", bufs=1) as const_pool,
            tc.tile_pool(name="xin", bufs=3) as x_pool,
            tc.tile_pool(name="zbuf", bufs=1) as z_pool,
            tc.tile_pool(name="bbuf", bufs=1) as b_pool,
            tc.tile_pool(name="z2buf", bufs=4) as z2_pool,
            tc.tile_pool(name="obuf", bufs=3) as o_pool,
            tc.tile_pool(name="ps_t0", bufs=2, space="PSUM") as t0_psum,
            tc.tile_pool(name="ps_a", bufs=2, space="PSUM") as a_psum,
            tc.tile_pool(name="ps_t1", bufs=2, space="PSUM") as t1_psum,
            tc.tile_pool(name="ps_b", bufs=2, space="PSUM") as b_psum,
        ):
            h128_t = const_pool.tile([128, 128], F32R)
            m32_t = const_pool.tile([128, 128], F32R)
            id_t = const_pool.tile([128, 128], F32R)
            nc.sync.dma_start(h128_t[:], h128_d)
            nc.sync.dma_start(m32_t[:], m32_d)
            nc.sync.dma_start(id_t[:], id_d)
            h128_r = h128_t[:]
            m32_r = m32_t[:]

            import contextlib

            loop_ctx = (
                tc.For_i(0, repeat) if hw_loop and repeat > 1
                else contextlib.nullcontext()
            )
            with loop_ctx:
              for b in range(N_BLOCKS * (1 if hw_loop else repeat)):
                  r0 = (b % N_BLOCKS) * BLOCK_ROWS

                  # ---- load 2 x [128, 4096] row subtiles ----
                  xt = []
                  for rs in range(2):
                      t = x_pool.tile([128, DIM], F32R)
                      eng = nc.sync if rs == 0 else nc.scalar
                      eng.dma_start(
                          t[:], x_d[r0 + rs * 128 : r0 + (rs + 1) * 128, :]
                      )
                      xt.append(t)

                  # ---- T0: Z[kc, c*256 + r_local], r_local = rs*128 + i ----
                  z = z_pool.tile([128, 32 * 256], F32R)
                  for cp in range(16):
                      ps = t0_psum.tile([128, 512], F32R)
                      for k in range(2):
                          c = cp * 2 + k
                          for rs in range(2):
                              nc.tensor.transpose(
                                  ps[:, k * 256 + rs * 128 : k * 256 + (rs + 1) * 128],
                                  xt[rs][:, c * 128 : (c + 1) * 128],
                                  id_t[:],
                              )
                      # split copies between ACT and DVE for load balance
                      if cp % 2 == 0:
                          nc.scalar.copy(z[:, cp * 512 : (cp + 1) * 512], ps[:])
                      else:
                          nc.vector.tensor_copy(z[:, cp * 512 : (cp + 1) * 512], ps[:])

                  # ---- stage A: B[kc', r_local*32 + c] (scatter per-c so T1
                  # reads contiguous 128-slices; PE rhs allows only 1 free dim)
                  bb = b_pool.tile([128, 32 * 256], F32R)
                  bb_sc = bb[:].rearrange("p (r c) -> p c r", c=32)
                  for cp in range(16):
                      ps = a_psum.tile([128, 512], F32)
                      nc.tensor.matmul(
                          ps[:], h128_r, z[:, cp * 512 : (cp + 1) * 512]
                      )
                      for k in range(2):
                          c = cp * 2 + k
                          if cp % 2 == 0:
                              nc.scalar.copy(
                                  bb_sc[:, c], ps[:, k * 256 : (k + 1) * 256]
                              )
                          else:
                              nc.vector.tensor_copy(
                                  bb_sc[:, c], ps[:, k * 256 : (k + 1) * 256]
                              )

                  out_halves = [
                      o_pool.tile(
                          [128, 16 * 256], F32, name=f"out_half_{b}_{i}", tag="out_half"
                      )
                      for i in range(2)
                  ]
                  for wp in range(16):
                      out = out_halves[wp // 8]
                      wo = wp % 8  # wp within half
                      # ---- T1: Z2[(rr,c), (w2,u,kc')] for w pair ----
                      ps = t1_psum.tile([128, 512], F32R)
                      for k in range(2):
                          w = wp * 2 + k
                          for u in range(2):
                              f0 = (w * 8 + u * 4) * 32
                              nc.tensor.transpose(
                                  ps[:, k * 256 + u * 128 : k * 256 + (u + 1) * 128],
                                  bb[:, f0 : f0 + 128],
                                  id_t[:],
                              )
                      z2 = z2_pool.tile([128, 512], F32R)
                      nc.scalar.copy(z2[:], ps[:])

                      # ---- stage B ----
                      psb = b_psum.tile([128, 512], F32)
                      nc.tensor.matmul(psb[:], m32_r, z2[:])
                      nc.vector.tensor_copy(out[:, wo * 512 : (wo + 1) * 512], psb[:])

                      # ---- DMA out after each half: [(rr',c'), (w,u,kc')] -> y ----
                      if wo == 7:
                          h = wp // 8
                          yb = y_d[r0 : r0 + BLOCK_ROWS, :].rearrange(
                              "(w u rr) (cp kc) -> rr cp w u kc", w=32, u=2, rr=4, cp=32
                          )
                          ob = out[:].rearrange("p (w u kc) -> p w u kc", w=16, u=2)
                          w0 = h * 16
                          nc.sync.dma_start(
                              yb[:, :, w0 : w0 + 8], ob[:, 0:8]
                          )
                          nc.scalar.dma_start(
                              yb[:, :, w0 + 8 : w0 + 16], ob[:, 8:16]
                          )

    nc.compile()
    return nc


_CACHE = {}


def _get_program():
    if "nc" not in _CACHE:
        _CACHE["nc"] = build_program()
    return _CACHE["nc"]


def kernel(x: np.ndarray, _trace: bool = False, _trace_kwargs=None) -> np.ndarray:
    assert x.shape == (4, 4096, 4096), x.shape
    x_flat = np.ascontiguousarray(x.reshape(16384, DIM), dtype=np.float32)
    h128, m32, ident = _constants()

    in_maps = []
    for i in range(N_CORES):
        in_maps.append(
            {
                "x": x_flat[i * ROWS_PER_CORE : (i + 1) * ROWS_PER_CORE],
                "h128": h128,
                "m32": m32,
                "ident": ident,
            }
        )

    nc = _get_program()
    res = bass_utils.run_bass_kernel_spmd(
        nc,
        in_maps,
        core_ids=list(range(N_CORES)),
        trace=_trace,
        **(_trace_kwargs or {}),
    )
    outs = [res.results[i]["y"] for i in range(N_CORES)]
    y = np.concatenate(outs, axis=0).reshape(4, 4096, 4096)
    if _trace:
        _CACHE["last_result"] = res
    return y

